# revision 13
# baseline (speedup 1.0000x reference)
"""Trainium2 Bass kernel for a GAT-style attention head.

Reference computation (B=1, C=512, N=8192, F=256):
    seq_fts = einsum('bcn,fc->bfn', x, W1)                  # [1,F,N]
    f1 = seq_fts . w21 + b21 ;  f2 = seq_fts . w22 + b22    # [1,N]
    logits[i,j] = f1[j] + f2[i]  masked by adj>0 (else -1e9)
    logits = leaky_relu(logits, 0.01)
    coefs = softmax(logits, axis=1)        # normalises over i for each j
    ret[i,f] = sum_j coefs[i,j]*seq_fts[f,j] + bias[f]
    out = elu(ret).transpose -> [1,F,N]

Distribution: shard rows i across 8 NeuronCores (1024 rows each).  E is
held transposed ([j on partitions, i free], fp16); the softmax
denominator D[j] = sum_i E[j,i] is chunk-AllReduced and folded into the
seq_fts columns before the local matmul out[f,i] = sum_j seqd[j,f]E[j,i].

Elementwise phase (the bottleneck) runs three tile pipelines interleaved
to balance ScalarE and VectorE:
  EXACT:  u0 = f2 + addmask (V TT, fp16 2x) ; lr = Prelu(u0 + f1[j])
          (S, fused bias+leaky-relu); E = Exp(lr) + accum D (S).
  SINGLE: lr = Prelu(f2 + f1[j]) (S, unmasked); fp16-bit fast-exp
          v = int16(A*lr + B) (V TS, 2x); E,D = TTR(bitcast(v) * mask01)
          (V, fused mask-mult + fp16 store + D accumulate).
  AVG:    same but two bit-exps with offset biases averaged in the TTR
          (halves the log-linear sawtooth error); mask additive pre-Prelu
          (masked lr ~= -10 -> bits land in the fp16 subnormal range ~ 0).
"""

import os
import sys

if "/opt/trn_rl_repo" not in sys.path:
    sys.path.insert(0, "/opt/trn_rl_repo")

import numpy as np

import concourse.bass as bass
import concourse.tile as tile
from concourse import bacc, mybir

F32 = mybir.dt.float32
F16 = mybir.dt.float16
F8 = mybir.dt.float8e4
I16 = mybir.dt.int16

B, C, N, F = 1, 512, 8192, 256
NCORES = 8
NB = N // NCORES          # rows per core (i block)
P = 128
NJT = N // P              # 64 j tiles
NS = NB // P              # 8 n sub-tiles per core
CO = C // P               # 4 contraction tiles
NCHUNK = 5
CH = [4, 20, 20, 16, 4]
CH0 = [0, 4, 24, 44, 60]
CJT = 20
SEQT_BLOCKS = [[0], [1, 2], [3, 4, 5], [6, 7], []]
MM_N = 512

# ---- elementwise pipeline mix (tuned against engine busy) ----
N_SINGLE = 44             # V-heavy fast-exp tiles (one sigma)
N_AVG = 0                 # V-heavy fast-exp tiles (two-sigma average)
# rest = exact S-path tiles
MASK_ADD = -1000.0        # additive mask: lrelu -> -10, exp/bit-exp -> ~0
A_SCH = 1477.3195645      # 1024*log2(e)
B_SCH = 15360.0 - 45.0    # fp16 exponent bias + sawtooth correction
DELTA = 180.0             # avg variant: biases B-DELTA, B+DELTA
S_SC = 6144.0             # seqd scale (keeps seq*S/D in fp16 normal range)

AF = mybir.ActivationFunctionType
OP = mybir.AluOpType


def _tile_kinds():
    """Assign a pipeline kind to each j-tile, interleaved evenly."""
    kinds = ["X"] * NJT            # exact by default
    nv = N_SINGLE + N_AVG
    vidx = [(k * NJT) // nv for k in range(nv)]
    for pos, k in enumerate(vidx):
        kinds[k] = "S" if (pos * N_SINGLE) // nv != ((pos + 1) * N_SINGLE) // nv else "A"
    return kinds


KINDS = _tile_kinds()
# per-kind row index within the packed per-kind mask tensors
MK01_IDX, MKA_IDX = {}, {}
for _jt, _k in enumerate(KINDS):
    if _k == "S":
        MK01_IDX[_jt] = len(MK01_IDX)
    else:
        MKA_IDX[_jt] = len(MKA_IDX)
N_MK01 = len(MK01_IDX)
N_MKA = len(MKA_IDX)

_PROGRAM_CACHE = {}
LAST_RESULTS = None


def _build_program(b21f: float, b22f: float):
    nc = bacc.Bacc("TRN2", target_bir_lowering=False, debug=False,
                   num_devices=NCORES)

    xb_t = nc.dram_tensor("xb", [C, NB], F32, kind="ExternalInput")
    w1t_t = nc.dram_tensor("w1t", [C, F], F32, kind="ExternalInput")
    w21_t = nc.dram_tensor("w21", [1, F], F32, kind="ExternalInput")
    w22_t = nc.dram_tensor("w22", [1, F], F32, kind="ExternalInput")
    bias_t = nc.dram_tensor("bias", [F], F32, kind="ExternalInput")
    id_t = nc.dram_tensor("ident", [P, P], F32, kind="ExternalInput")
    mk01_t = nc.dram_tensor("mk01", [max(N_MK01, 1) * P, NB], F8,
                            kind="ExternalInput")
    mka_t = nc.dram_tensor("mka", [max(N_MKA, 1) * P, NB], F16,
                           kind="ExternalInput")
    out_t = nc.dram_tensor("outb", [F, NB], F32, kind="ExternalOutput")

    groups = [list(range(NCORES))]

    with tile.TileContext(nc) as tc:
        with tc.tile_pool(name="dram", bufs=1, space="DRAM") as dram:
            ag1_in = dram.tile([NB], F32, name="ag1_in")
            ag1_out = dram.tile([N], F32, name="ag1_out", addr_space="Shared")
            ag2_in = dram.tile([NB * F], F16, name="ag2_in")
            ag2_out = dram.tile([N * F], F16, name="ag2_out",
                                addr_space="Shared")
            f2tmp = dram.tile([NB], F16, name="f2tmp")
            ar_in = [dram.tile([P * CH[k]], F32, name=f"ar_in{k}")
                     for k in range(NCHUNK)]
            ar_out = [dram.tile([P * CH[k]], F32, name=f"ar_out{k}",
                                addr_space="Shared") for k in range(NCHUNK)]

            with tc.tile_pool(name="persist", bufs=1) as persist:
                seqt = persist.tile([P, NJT, F], F16, name="seqt")
                f2b16 = persist.tile([P, NB], F16, name="f2b16")
                f1col = persist.tile([P, NJT], F32, name="f1col")
                bias_sb = persist.tile([P, F // P], F32, name="bias_sb")
                ident = persist.tile([P, P], F32, name="ident")

                # ---------- phase 0 ----------
                with tc.tile_pool(name="p0", bufs=1) as p0, \
                     tc.tile_pool(name="p0ps", bufs=2, space="PSUM") as p0ps:
                    nc.sync.dma_start(ident[:], id_t.ap())
                    x_sb = p0.tile([P, CO, NB], F32, name="x_sb")
                    nc.sync.dma_start(
                        x_sb[:],
                        xb_t.ap().rearrange("(co ci) n -> ci co n", ci=P))
                    w1t_sb = p0.tile([P, CO, F], F32, name="w1t_sb")
                    nc.sync.dma_start(
                        w1t_sb[:],
                        w1t_t.ap().rearrange("(co ci) f -> ci co f", ci=P))
                    w21b = p0.tile([P, F], F32, name="w21b")
                    nc.sync.dma_start(w21b[:],
                                      w21_t.ap()[0:1, :].to_broadcast((P, F)))
                    w22b = p0.tile([P, F], F32, name="w22b")
                    nc.sync.dma_start(w22b[:],
                                      w22_t.ap()[0:1, :].to_broadcast((P, F)))
                    nc.sync.dma_start(
                        bias_sb[:],
                        bias_t.ap().rearrange("(ft fi) -> fi ft", fi=P))

                    # u1/u2 = W1^T w21 / w22  (fp32, c on partitions)
                    u_sb = p0.tile([P, CO, 2], F32, name="u_sb")
                    for co in range(CO):
                        tu = p0.tile([P, F], F32, name="tu", tag="tu")
                        nc.vector.tensor_tensor(tu[:], w1t_sb[:, co, :],
                                                w21b[:], OP.mult)
                        nc.vector.tensor_reduce(u_sb[:, co, 0:1], tu[:],
                                                mybir.AxisListType.X, OP.add)
                        tv = p0.tile([P, F], F32, name="tv", tag="tv")
                        nc.vector.tensor_tensor(tv[:], w1t_sb[:, co, :],
                                                w22b[:], OP.mult)
                        nc.vector.tensor_reduce(u_sb[:, co, 1:2], tv[:],
                                                mybir.AxisListType.X, OP.add)

                    # fp16 casts (shared by f1/f2 and seqT matmuls)
                    xh = p0.tile([P, CO, NB], F16, name="xh")
                    nc.vector.tensor_copy(xh[:], x_sb[:])
                    w1h = p0.tile([P, CO, F], F16, name="w1h")
                    nc.vector.tensor_copy(w1h[:], w1t_sb[:])
                    uh = p0.tile([P, CO, 2], F16, name="uh")
                    nc.vector.tensor_copy(uh[:], u_sb[:])

                    # f1/f2 via fp16 matmul. f1/f2 errors are per-row /
                    # per-column scales on E that (nearly) cancel in the
                    # softmax, so single-term fp16 is plenty; f1 first so
                    # its AllGather (which gates the elementwise phase)
                    # starts as early as possible.
                    f1ps = p0ps.tile([1, NB], F32, name="f1ps", bufs=1)
                    f2ps = p0ps.tile([1, NB], F32, name="f2ps", bufs=1)
                    for q, ps in ((0, f1ps), (1, f2ps)):
                        for ih in range(2):
                            sl = slice(ih * MM_N, (ih + 1) * MM_N)
                            for co in range(CO):
                                nc.tensor.matmul(
                                    ps[:, sl], lhsT=uh[:, co, q:q + 1],
                                    rhs=xh[:, co, sl],
                                    start=(co == 0), stop=(co == CO - 1))
                        if q == 0:
                            f1row = p0.tile([1, NB], F32, name="f1row")
                            nc.vector.tensor_scalar_add(f1row[:], f1ps[:],
                                                        b21f)
                            nc.sync.dma_start(
                                ag1_in[:].rearrange("n -> () n"), f1row[:])
                            # small AllGather: f1 (32KB total)
                            nc.gpsimd.collective_compute(
                                "AllGather", OP.bypass, replica_groups=groups,
                                ins=[ag1_in.opt()], outs=[ag1_out.opt()])
                    f2row = p0.tile([1, NB], F16, name="f2row")
                    nc.vector.tensor_scalar_add(f2row[:], f2ps[:], b22f)
                    nc.sync.dma_start(f2tmp[:].rearrange("n -> () n"),
                                      f2row[:])
                    nc.sync.dma_start(
                        f2b16[:], f2tmp[None, :].to_broadcast((P, NB)))

                    # f1col[jp, jt] = f1[jt*128+jp] via PE transpose
                    t64 = p0.tile([NJT, P], F32, name="t64")
                    nc.gpsimd.dma_start(
                        t64[:], ag1_out.rearrange("(jt jp) -> jt jp", jp=P))
                    tps = p0ps.tile([P, NJT], F32, name="tps", bufs=1)
                    nc.tensor.matmul(tps[:], lhsT=t64[:],
                                     rhs=ident[:NJT, :NJT],
                                     is_transpose=True, start=True, stop=True)
                    nc.scalar.copy(f1col[:], tps[:])

                    # seqT (own block) and its AllGather
                    seqtown = p0.tile([P, NS, F], F16, name="seqtown")
                    for ns in range(NS):
                        sps = p0ps.tile([P, F], F32, name="sps", tag="sps")
                        for co in range(CO):
                            nc.tensor.matmul(
                                sps[:],
                                lhsT=xh[:, co, ns * P:(ns + 1) * P],
                                rhs=w1h[:, co, :],
                                start=(co == 0), stop=(co == CO - 1))
                        nc.vector.tensor_copy(seqtown[:, ns, :], sps[:])
                    nc.sync.dma_start(
                        ag2_in.rearrange("(ci ns f) -> ci ns f",
                                         ci=P, ns=NS),
                        seqtown[:])
                    nc.gpsimd.collective_compute(
                        "AllGather", OP.bypass, replica_groups=groups,
                        ins=[ag2_in.opt()], outs=[ag2_out.opt()])

                # ---------- main loop ----------
                with tc.tile_pool(name="etpool", bufs=1) as etp, \
                     tc.tile_pool(name="stream", bufs=3) as stream, \
                     tc.tile_pool(name="dtiles", bufs=1) as dtiles, \
                     tc.tile_pool(name="outps", bufs=1, space="PSUM") as outps, \
                     tc.tile_pool(name="epil", bufs=1) as epil:

                    out_ps = [outps.tile([P, MM_N], F32, name=f"out_ps{q}",
                                         tag=f"out_ps{q}")
                              for q in range(4)]
                    dp_c = [dtiles.tile([P, CH[k]], F32, name=f"dp{k}",
                                        tag=f"dp{k}") for k in range(NCHUNK)]
                    inv_c = [dtiles.tile([P, CH[k]], F32, name=f"inv{k}",
                                         tag=f"inv{k}") for k in range(NCHUNK)]
                    et_k = [None] * NCHUNK

                    def elem_chunk(k):
                        et = etp.tile([P, CJT, NB], F16, name=f"et{k % 2}",
                                      tag=f"et{k % 2}")
                        et_k[k] = et
                        for jl in range(CH[k]):
                            jt = CH0[k] + jl
                            kind = KINDS[jt]
                            if kind == "X":
                                mka = stream.tile([P, NB], F16, name="mka",
                                                  tag="mka", bufs=6)
                                nc.sync.dma_start(
                                    mka[:],
                                    mka_t.ap()[MKA_IDX[jt] * P:
                                               (MKA_IDX[jt] + 1) * P, :])
                                u0 = stream.tile([P, NB], F16, name="u0",
                                                 tag="u0", bufs=3)
                                nc.vector.tensor_tensor(u0[:], f2b16[:],
                                                        mka[:], OP.add)
                                lr = stream.tile([P, NB], F16, name="lr",
                                                 tag="lrx", bufs=3)
                                nc.scalar.activation(
                                    lr[:], u0[:], AF.Prelu,
                                    bias=f1col[:, jt:jt + 1], scale=1.0,
                                    alpha=0.01)
                                nc.scalar.activation(
                                    et[:, jl, :], lr[:], AF.Exp,
                                    accum_out=dp_c[k][:, jl:jl + 1])
                            elif kind == "S":
                                mk8 = stream.tile([P, NB], F8, name="mk8",
                                                  tag="mk8", bufs=4)
                                nc.sync.dma_start(
                                    mk8[:],
                                    mk01_t.ap()[MK01_IDX[jt] * P:
                                                (MK01_IDX[jt] + 1) * P, :])
                                lr = stream.tile([P, NB], F16, name="lrs",
                                                 tag="lrs", bufs=3)
                                nc.scalar.activation(
                                    lr[:], f2b16[:], AF.Prelu,
                                    bias=f1col[:, jt:jt + 1], scale=1.0,
                                    alpha=0.01)
                                va = stream.tile([P, NB], I16, name="va",
                                                 tag="va", bufs=2)
                                nc.vector.tensor_scalar(
                                    va[:], lr[:], A_SCH, B_SCH,
                                    OP.mult, OP.add)
                                nc.vector.scalar_tensor_tensor(
                                    et[:, jl, :], va[:].bitcast(F16), 1.0,
                                    mk8[:], OP.mult, OP.mult,
                                    accum_out=dp_c[k][:, jl:jl + 1])
                            else:  # AVG
                                mka = stream.tile([P, NB], F16, name="mka",
                                                  tag="mka", bufs=6)
                                nc.sync.dma_start(
                                    mka[:],
                                    mka_t.ap()[MKA_IDX[jt] * P:
                                               (MKA_IDX[jt] + 1) * P, :])
                                u0 = stream.tile([P, NB], F16, name="u0",
                                                 tag="u0", bufs=3)
                                nc.vector.tensor_tensor(u0[:], f2b16[:],
                                                        mka[:], OP.add)
                                lr = stream.tile([P, NB], F16, name="lra",
                                                 tag="lra", bufs=3)
                                nc.scalar.activation(
                                    lr[:], u0[:], AF.Prelu,
                                    bias=f1col[:, jt:jt + 1], scale=1.0,
                                    alpha=0.01)
                                # bits-1024 halves an fp16 value, so each
                                # term carries the /2 of the average
                                va = stream.tile([P, NB], I16, name="vaa",
                                                 tag="vaa", bufs=2)
                                nc.vector.tensor_scalar(
                                    va[:], lr[:], A_SCH,
                                    B_SCH - DELTA - 1024.0,
                                    OP.mult, OP.add)
                                vb = stream.tile([P, NB], I16, name="vab",
                                                 tag="vab", bufs=2)
                                nc.vector.tensor_scalar(
                                    vb[:], lr[:], A_SCH,
                                    B_SCH + DELTA - 1024.0,
                                    OP.mult, OP.add)
                                nc.vector.scalar_tensor_tensor(
                                    et[:, jl, :], va[:].bitcast(F16), 1.0,
                                    vb[:].bitcast(F16), OP.mult, OP.add,
                                    accum_out=dp_c[k][:, jl:jl + 1])
                        # chunk-k AllReduce of partial D
                        nc.gpsimd.dma_start(
                            ar_in[k].rearrange("(jp jl) -> jp jl", jp=P),
                            dp_c[k][:])
                        nc.gpsimd.collective_compute(
                            "AllReduce", OP.add, replica_groups=groups,
                            ins=[ar_in[k].opt()], outs=[ar_out[k].opt()])

                    def consume_chunk(k):
                        dsum = dtiles.tile([P, CH[k]], F32, name=f"dsum{k}",
                                           tag=f"dsum{k}")
                        nc.gpsimd.dma_start(
                            dsum[:],
                            ar_out[k].rearrange("(jp jl) -> jp jl", jp=P))
                        nc.vector.reciprocal(inv_c[k][:], dsum[:])
                        srcv = ag2_out.rearrange(
                            "(b ci ns f) -> b ci ns f", b=NCORES, ci=P, ns=NS)
                        for b in SEQT_BLOCKS[k]:
                            nc.sync.dma_start(seqt[:, b * NS:(b + 1) * NS, :],
                                              srcv[b])
                        seqds = []
                        for jl in range(CH[k]):
                            jt = CH0[k] + jl
                            sd = stream.tile([P, F], F16, name="seqd",
                                             tag="seqd", bufs=12)
                            nc.vector.tensor_scalar(
                                sd[:], seqt[:, jt, :],
                                inv_c[k][:, jl:jl + 1], S_SC,
                                OP.mult, OP.mult)
                            seqds.append(sd)
                        et = et_k[k]
                        for jl in range(CH[k]):
                            jt = CH0[k] + jl
                            for fi in range(2):
                                for ih in range(2):
                                    nc.tensor.matmul(
                                        out_ps[fi * 2 + ih][:],
                                        lhsT=seqds[jl][:,
                                                  fi * P:(fi + 1) * P],
                                        rhs=et[:, jl,
                                               ih * MM_N:(ih + 1) * MM_N],
                                        start=(jt == 0), stop=(jt == NJT - 1))

                    # software pipeline: defer consume(k) until after
                    # elem(k+1) is emitted so the AllReduce latency hides
                    # behind the next chunk's elementwise work
                    elem_chunk(0)
                    for k in range(1, NCHUNK):
                        elem_chunk(k)
                        consume_chunk(k - 1)
                    consume_chunk(NCHUNK - 1)

                    # ---------- epilogue: bias + ELU ----------
                    # elu(t) = relu(t) + exp(min(t,0)) - 1
                    for fi in range(2):
                        for ih in range(2):
                            ps = out_ps[fi * 2 + ih]
                            t = epil.tile([P, MM_N], F32, name="t",
                                          tag="ep_t", bufs=2)
                            nc.scalar.activation(t[:], ps[:], AF.Identity,
                                                 bias=bias_sb[:, fi:fi + 1],
                                                 scale=1.0 / S_SC)
                            m = epil.tile([P, MM_N], F32, name="m",
                                          tag="ep_m", bufs=2)
                            nc.vector.tensor_scalar_min(m[:], t[:], 0.0)
                            nc.vector.tensor_scalar_max(t[:], t[:], 0.0)
                            nc.scalar.activation(m[:], m[:], AF.Exp)
                            nc.vector.scalar_tensor_tensor(
                                m[:], m[:], -1.0, t[:], OP.add, OP.add)
                            nc.sync.dma_start(
                                out_t.ap()[fi * P:(fi + 1) * P,
                                           ih * MM_N:(ih + 1) * MM_N], m[:])

    nc.compile()
    return nc


def kernel(x, adj, W1, w21, b21, w22, b22, bias):
    global LAST_RESULTS
    from concourse.bass_utils import run_bass_kernel_spmd
    import ml_dtypes

    x = np.asarray(x)
    adj = np.asarray(adj)
    W1 = np.asarray(W1, dtype=np.float32)
    w21 = np.asarray(w21, dtype=np.float32)
    w22 = np.asarray(w22, dtype=np.float32)
    bias = np.asarray(bias, dtype=np.float32)
    b21f = float(np.asarray(b21))
    b22f = float(np.asarray(b22))

    key = (b21f, b22f)
    if key not in _PROGRAM_CACHE:
        _PROGRAM_CACHE[key] = _build_program(b21f, b22f)
    nc = _PROGRAM_CACHE[key]

    w1t = np.ascontiguousarray(W1.T)
    identity = np.eye(P, dtype=np.float32)
    in_maps = []
    for c in range(NCORES):
        blk = slice(c * NB, (c + 1) * NB)
        xb = np.ascontiguousarray(x[0, :, blk], dtype=np.float32)
        adjT = adj[0, blk, :].T                     # [j, i_local] int32
        # multiplicative {0,1} fp8 rows for SINGLE tiles
        mk01 = np.empty((max(N_MK01, 1) * P, NB), dtype=ml_dtypes.float8_e4m3)
        mka = np.empty((max(N_MKA, 1) * P, NB), dtype=np.float16)
        for jt, kind in enumerate(KINDS):
            rows = adjT[jt * P:(jt + 1) * P, :]
            if kind == "S":
                r = MK01_IDX[jt]
                mk01[r * P:(r + 1) * P, :] = rows.astype(
                    ml_dtypes.float8_e4m3)
            else:
                r = MKA_IDX[jt]
                mka[r * P:(r + 1) * P, :] = (
                    (rows.astype(np.float32) - 1.0) * (-MASK_ADD)
                ).astype(np.float16)
        in_maps.append({
            "xb": xb,
            "ident": identity,
            "w1t": w1t,
            "w21": w21.reshape(1, F),
            "w22": w22.reshape(1, F),
            "bias": bias,
            "mk01": mk01,
            "mka": mka,
        })

    trace = os.environ.get("BASS_KERNEL_TRACE") == "1"
    kwargs = {}
    if trace:
        _install_ntff_hook()
        import concourse.bass_utils as bu
        bu.upload_artifacts = lambda d: d
        kwargs = dict(trace=True, trace_cores=list(range(NCORES)),
                      tmpdir=os.environ.get("BASS_KERNEL_TRACE_DIR"))

    res = run_bass_kernel_spmd(nc, in_maps, core_ids=list(range(NCORES)),
                               **kwargs)
    LAST_RESULTS = res

    out = np.empty((B, F, N), dtype=np.float32)
    for c in range(NCORES):
        out[0, :, c * NB:(c + 1) * NB] = res.results[c]["outb"]
    return out


def _install_ntff_hook():
    """Register the axon NTFF profiling hook (missing antenv.axon_hooks)."""
    import types
    import contextlib
    import ctypes

    if "antenv.axon_hooks" in sys.modules:
        return
    so_path = "/opt/axon/libaxon_pjrt.so"
    lib = ctypes.CDLL(so_path)
    if not hasattr(lib, "axon_start_nrt_profile"):
        return
    lib.axon_start_nrt_profile.argtypes = [ctypes.POINTER(ctypes.c_int64),
                                           ctypes.c_size_t]
    lib.axon_start_nrt_profile.restype = ctypes.c_int64
    lib.axon_stop_nrt_profile.argtypes = [ctypes.c_char_p]
    lib.axon_stop_nrt_profile.restype = ctypes.c_int64

    @contextlib.contextmanager
    def _hook(output_dir, device_ids):
        import jax
        jax.devices()
        if device_ids:
            ids = (ctypes.c_int64 * len(device_ids))(*device_ids)
            rc = lib.axon_start_nrt_profile(ids, len(device_ids))
        else:
            rc = lib.axon_start_nrt_profile(None, 0)
        if rc != 0:
            raise RuntimeError(f"axon_start_nrt_profile rc={rc}")
        try:
            yield
        finally:
            n = lib.axon_stop_nrt_profile(str(output_dir).encode())
            print(f"ntff profile: {n} file(s) -> {output_dir}",
                  file=sys.stderr)

    mod = types.ModuleType("antenv.axon_hooks")
    mod.get_axon_ntff_profile_hook = lambda: _hook
    mod.set_axon_ntff_profile_hook = lambda h: None
    sys.modules["antenv.axon_hooks"] = mod
